# revision 6
# baseline (speedup 1.0000x reference)
"""Trainium2 Bass kernel for nn_CrossAttention_82471962018390.

Dilated (d=2) 9x9 neighborhood cross-attention, q 48x48 vs k/v 24x24.

Math identity used: the nearest-exact 2x upsample + dilation-2 NATTEN window
collapses so that query (h, w) attends to the ORIGINAL 24x24 k/v grid at
rows clip(h//2-4, 0, 15) + 0..8, cols clip(w//2-4, 0, 15) + 0..8 (a
contiguous 9x9 window; the 4 queries in each 2x2 block share one window).

Kernel structure (per (b, head) pair; 2 pairs per core, 8 cores = 16 pairs):
  - 16 row-bands by s_h = clip(h//2-4,0,15); band s attends the 9x24=216-key
    slab k[:, s:s+9, :].
  - Scores computed transposed: S^T[key, query] = (K slab)^T @ Q, with the
    column-window mask folded INTO the matmul via 16 extra contraction rows:
    lhsT rows 64:80 hold M0[r, kw] (0 or -30), rhs rows 64:80 hold the
    one-hot of s_w(w).  scale 1/8 is folded into q on the host.
  - Both key chunks (128 + 88) land in ONE [128, 1024] PSUM tile (chunk2 at
    column offset 512), so a single wide ScalarE exp covers the whole group.
  - One PV matmul pair per band with stationary [V^T | ones*64] giving
    [PV (64 rows); sumexp replicated (64 rows)] in one PSUM tile.
  - reciprocal_approx_fast (DVE, ~5x faster than exact reciprocal) on the
    replicated sumexp, then a tensor multiply alternating between DVE and
    GpSimd to balance engines; DMA out.
Bands are grouped (10/2*5/2*5/2*4/10 h-rows) so ACT/DVE instructions run on
~480-column tiles.  The issue order is software-pipelined: QK runs two
iterations ahead of PV so the PE never waits for the exp.
"""

import numpy as np
import ml_dtypes

try:
    import concourse.bass as bass
    import concourse.bacc as bacc
    import concourse.tile as tile
    from concourse import mybir
    from concourse.bass_utils import run_bass_kernel_spmd
except ImportError:  # pragma: no cover
    import sys

    sys.path.insert(0, "/opt/trn_rl_repo")
    import concourse.bass as bass
    import concourse.bacc as bacc
    import concourse.tile as tile
    from concourse import mybir
    from concourse.bass_utils import run_bass_kernel_spmd

from contextlib import ExitStack

BF = ml_dtypes.bfloat16
N_CORES = 8
NPAIR = 2  # (b, head) pairs per core
DH = 64
HQ = WQ = 48
HK = WK = 24
NQ = HQ * WQ  # 2304
NK = HK * WK  # 576
BAND_KEYS = 9 * WK  # 216
CH1 = 128  # keys in chunk 1 of a band
CH2 = BAND_KEYS - CH1  # 88
C2OFF = 512  # column offset of chunk-2 scores inside the PSUM score tile

# s(i) = clip(i//2 - 4, 0, 15) for i in 0..47
_S = np.clip(np.arange(48) // 2 - 4, 0, 15)

# Band groups: (q column offset, width, [(band s, rel q offset, band width)])
# band s covers h rows where s_h(h) == s: s=0 -> h 0..9, s=1..14 -> 2 rows,
# s=15 -> h 38..47.
def _groups():
    bands_h0 = {}
    for h in range(48):
        bands_h0.setdefault(int(_S[h]), []).append(h)
    spans = {s: (hs[0], len(hs)) for s, hs in bands_h0.items()}
    layout = [[0], [1, 2, 3, 4, 5], [6, 7, 8, 9, 10], [11, 12, 13, 14], [15]]
    groups = []
    for g in layout:
        h0 = spans[g[0]][0]
        width = sum(spans[s][1] for s in g) * 48
        bands = []
        off = 0
        for s in g:
            bw = spans[s][1] * 48
            bands.append((s, off, bw))
            off += bw
        groups.append((h0 * 48, width, bands))
    return groups


GROUPS = _groups()
MAXM = max(w for _, w, _ in GROUPS)  # 480


def _host_tables():
    """M0exp [16, 576] and Bw [16, 2304] mask/one-hot tables (fp32)."""
    m0 = np.full((16, WK), -30.0, np.float32)
    for r in range(16):
        m0[r, r : r + 9] = 0.0
    m0exp = np.tile(m0[:, None, :], (1, HK, 1)).reshape(16, NK)
    bw = np.zeros((16, NQ), np.float32)
    for w in range(48):
        bw[_S[w], np.arange(48) * 48 + w] = 1.0
    return m0exp, bw


def build_kernel(ctx: ExitStack, tc, qb, km, vb, out):
    nc = tc.nc
    FP32 = mybir.dt.float32
    BF16 = mybir.dt.bfloat16
    Exp = mybir.ActivationFunctionType.Exp

    qpool = ctx.enter_context(tc.tile_pool(name="qb", bufs=1))
    kpool = ctx.enter_context(tc.tile_pool(name="km", bufs=1))
    vpool = ctx.enter_context(tc.tile_pool(name="vt", bufs=1))
    spool = ctx.enter_context(tc.tile_pool(name="scores", bufs=1, space="PSUM"))
    opool = ctx.enter_context(tc.tile_pool(name="opsum", bufs=1, space="PSUM"))
    epool = ctx.enter_context(tc.tile_pool(name="expo", bufs=1))
    rpool = ctx.enter_context(tc.tile_pool(name="res", bufs=1))

    ITERS = [(p, gi) for p in range(NPAIR) for gi in range(len(GROUPS))]
    NI = len(ITERS)

    # Persistent double-buffered tiles, managed explicitly for pipelining.
    sT = [spool.tile([128, 1024], FP32, name=f"s{j}") for j in range(2)]
    eT = [epool.tile([128, 992], BF16, name=f"e{j}") for j in range(2)]
    oT = [opool.tile([128, MAXM], FP32, name=f"o{j}") for j in range(2)]
    rcpT = [rpool.tile([64, MAXM], FP32, name=f"rcp{j}") for j in range(2)]
    resT = [rpool.tile([64, MAXM], FP32, name=f"res{j}") for j in range(2)]

    # Rotating persistent V'' stationary tiles: cols 0:64 = V^T chunk (DMA'd
    # per band), cols 64:128 = ones (memset once; gives replicated sumexp).
    NVT = 24
    vtiles = [vpool.tile([128, 128], BF16, name=f"vt{j}") for j in range(NVT)]
    for vt in vtiles:
        nc.gpsimd.memset(vt[:, 64:128], 1.0)
    # One-time init of never-matmul-written PSUM regions covered by the fused
    # exp reads: the 32-col gap between chunks and chunk2's unused rows.
    # (partition offsets must be 0/32/64/96: cover rows 64:128, the
    # chunk-2 matmuls overwrite rows 64:88 afterwards anyway)
    for s in sT:
        nc.vector.memset(s[:, 480:512], 0.0)
        nc.vector.memset(s[64:128, C2OFF : C2OFF + 512], 0.0)

    # Per-pair inputs: km in one DMA; qb split per group so QK(0) starts early.
    km_t, qb_t = [], []
    for p in range(NPAIR):
        kt = kpool.tile([80, NK], BF16, name=f"km{p}")
        nc.sync.dma_start(kt[:, :], km[80 * p : 80 * p + 80, :])
        km_t.append(kt)
        qt = qpool.tile([80, NQ], BF16, name=f"qb{p}")
        qb_t.append(qt)
    for gi, (q0, M, _) in enumerate(GROUPS):
        for p in range(NPAIR):
            nc.sync.dma_start(
                qb_t[p][:, q0 : q0 + M], qb[80 * p : 80 * p + 80, q0 : q0 + M]
            )

    # Pre-assign rotating V tiles per iteration (deterministic round-robin).
    vmap = []
    vct = 0
    for p, gi in ITERS:
        pairs = []
        for _ in GROUPS[gi][2]:
            pairs.append((vtiles[vct % NVT], vtiles[(vct + 1) % NVT]))
            vct += 2
        vmap.append(pairs)

    def vdma(i):
        p, gi = ITERS[i]
        for (s, _, _), (vta, vtb) in zip(GROUPS[gi][2], vmap[i]):
            row0 = (p * 16 + s) * BAND_KEYS
            nc.sync.dma_start(vta[:, 0:64], vb[row0 : row0 + CH1, :])
            nc.sync.dma_start(vtb[0:CH2, 0:64], vb[row0 + CH1 : row0 + BAND_KEYS, :])

    def qk(i):
        p, gi = ITERS[i]
        q0, M, bands = GROUPS[gi]
        s = sT[i % 2]
        for (b, off, bw) in bands:
            rhs = qb_t[p][:, q0 + off : q0 + off + bw]
            nc.tensor.matmul(
                s[:, off : off + bw],
                km_t[p][:, 24 * b : 24 * b + CH1],
                rhs,
                start=True,
                stop=True,
            )
            nc.tensor.matmul(
                s[0:CH2, C2OFF + off : C2OFF + off + bw],
                km_t[p][:, 24 * b + CH1 : 24 * b + BAND_KEYS],
                rhs,
                start=True,
                stop=True,
            )

    def expi(i):
        _, gi = ITERS[i]
        M = GROUPS[gi][1]
        s, e = sT[i % 2], eT[i % 2]
        # Two instructions: an ACT read must not cross a PSUM bank boundary.
        nc.scalar.activation(e[:, 0:M], s[:, 0:M], Exp)
        nc.scalar.activation(
            e[0:CH2, C2OFF : C2OFF + M], s[0:CH2, C2OFF : C2OFF + M], Exp
        )

    def pv(i):
        p, gi = ITERS[i]
        o = oT[i % 2]
        e = eT[i % 2]
        for (b, off, bw), (vta, vtb) in zip(GROUPS[gi][2], vmap[i]):
            # Same-output WAW dep keeps the accumulation pair ordered;
            # CoreSim's psum-group check validates the final schedule.
            nc.tensor.matmul(
                o[:, off : off + bw],
                vta[:, :],
                e[:, off : off + bw],
                start=True,
                stop=False,
            )
            nc.tensor.matmul(
                o[:, off : off + bw],
                vtb[0:CH2, :],
                e[0:CH2, C2OFF + off : C2OFF + off + bw],
                start=False,
                stop=True,
            )

    def norm(i):
        p, gi = ITERS[i]
        q0, M, _ = GROUPS[gi]
        o, rcp, res = oT[i % 2], rcpT[i % 2], resT[i % 2]
        # GPSIMD has no PSUM port, so both normalize steps stay on DVE.
        nc.vector.reciprocal(rcp[:, :M], o[64:128, :M])
        nc.vector.tensor_mul(res[:, :M], o[0:64, :M], rcp[:, :M])
        nc.sync.dma_start(out[64 * p : 64 * p + 64, q0 : q0 + M], res[:, :M])

    # Software-pipelined issue order: PE stays two iterations ahead on QK.
    vdma(0)
    vdma(1)
    qk(0)
    qk(1)
    expi(0)
    for i in range(NI):
        pv(i)
        if i + 2 < NI:
            vdma(i + 2)
            qk(i + 2)
        if i + 1 < NI:
            expi(i + 1)
        norm(i)


_CACHE = {}


def _get_nc():
    if "nc" not in _CACHE:
        nc = bacc.Bacc(
            "TRN2", target_bir_lowering=False, debug=False, num_devices=N_CORES
        )
        qb = nc.dram_tensor(
            "qb", [NPAIR * 80, NQ], mybir.dt.bfloat16, kind="ExternalInput"
        ).ap()
        km = nc.dram_tensor(
            "km", [NPAIR * 80, NK], mybir.dt.bfloat16, kind="ExternalInput"
        ).ap()
        vb = nc.dram_tensor(
            "vb", [NPAIR * 16 * BAND_KEYS, DH], mybir.dt.bfloat16, kind="ExternalInput"
        ).ap()
        out = nc.dram_tensor(
            "out", [NPAIR * 64, NQ], mybir.dt.float32, kind="ExternalOutput"
        ).ap()
        with tile.TileContext(nc) as tc, ExitStack() as ctx:
            build_kernel(ctx, tc, qb, km, vb, out)
        nc.compile()
        _CACHE["nc"] = nc
    return _CACHE["nc"]


def kernel(q: np.ndarray, k: np.ndarray, v: np.ndarray) -> np.ndarray:
    assert q.shape == (2, 512, HQ, WQ) and k.shape == (2, 512, HK, WK)
    m0exp, bw = _host_tables()
    nc = _get_nc()

    in_maps = []
    for c in range(N_CORES):
        qbc = np.empty((NPAIR * 80, NQ), BF)
        kmc = np.empty((NPAIR * 80, NK), BF)
        vbc = np.empty((NPAIR * 16 * BAND_KEYS, DH), BF)
        for pl in range(NPAIR):
            pg = NPAIR * c + pl
            b, hd = pg // 8, pg % 8
            qbc[80 * pl : 80 * pl + 64] = (
                q[b, 64 * hd : 64 * hd + 64].reshape(64, NQ) / 8.0
            ).astype(BF)
            qbc[80 * pl + 64 : 80 * pl + 80] = bw.astype(BF)
            kmc[80 * pl : 80 * pl + 64] = (
                k[b, 64 * hd : 64 * hd + 64].reshape(64, NK).astype(BF)
            )
            kmc[80 * pl + 64 : 80 * pl + 80] = m0exp.astype(BF)
            v3 = v[b, 64 * hd : 64 * hd + 64].reshape(64, HK, WK)
            for s in range(16):
                row0 = (pl * 16 + s) * BAND_KEYS
                vbc[row0 : row0 + BAND_KEYS] = (
                    v3[:, s : s + 9, :].reshape(64, BAND_KEYS).T.astype(BF)
                )
        in_maps.append({"qb": qbc, "km": kmc, "vb": vbc})

    results = run_bass_kernel_spmd(nc, in_maps, list(range(N_CORES))).results

    out = np.empty((2, 512, HQ, WQ), np.float32)
    for c in range(N_CORES):
        o = results[c]["out"]
        for pl in range(NPAIR):
            pg = NPAIR * c + pl
            b, hd = pg // 8, pg % 8
            out[b, 64 * hd : 64 * hd + 64] = o[64 * pl : 64 * pl + 64].reshape(
                64, HQ, WQ
            )
    return out


if __name__ == "__main__":
    qq = np.load("/root/problem/q.npy")
    kk = np.load("/root/problem/k.npy")
    vv = np.load("/root/problem/v.npy")
    got = kernel(qq, kk, vv)
    exp = np.load("/root/problem/expected.npy")
    rel = np.linalg.norm(got - exp) / np.linalg.norm(exp)
    print("Relative error:", rel)
